# revision 1
# baseline (speedup 1.0000x reference)
# Trainium2 Bass kernel for nn_AttentionalPropagation (B=2, D=256, N=M=4096, H=4).
#
# Sharding: 8 cores; each batch (B=2) owns 4 cores; each core computes a
# 1024-column sequence shard of the output end-to-end (q/scores/softmax/attn/
# message/MLP). k,v are computed redundantly per core from the full `source`
# of its batch. The only cross-core communication is an AllReduce of the
# InstanceNorm partial (sum, sumsq) statistics within each 4-core batch group.
#
# Per-head layout trick: conv weights' output channels are permuted host-side
# so that head channels are contiguous on SBUF partitions (head h lives at
# partitions 64*(h%2) of channel-chunk h//2); this lets per-head matmuls run
# directly off partition-aligned slices (PE row/col tiling).
#
# Softmax: scores are built transposed ([m, n] with m on partitions), exp'd on
# ScalarE (scale=1/8 folded in, no max-subtraction needed: |s/8| < ~5), and the
# softmax denominator is obtained for free by augmenting v^T with a ones
# column in the attn matmul (out row 64 = sum_m exp). Normalization is a
# per-column reciprocal broadcast multiply.

import numpy as np

import concourse.bass as bass  # noqa: F401  (bass types used via tile/bacc)
import concourse.tile as tile
import concourse.mybir as mybir
from concourse import bacc
from concourse import bass_utils

B, D, N = 2, 256, 4096
H, DH = 4, 64
NS = N // 4           # sequence shard per core
NCORES = 8
EPS = 1e-5

FP = mybir.dt.float32
BF = mybir.dt.bfloat16
F8 = mybir.dt.float8e4
AX = mybir.AxisListType
OP = mybir.AluOpType
AF = mybir.ActivationFunctionType

# j-groups for the scores->exp pipeline. Each group's PSUM tile holds BOTH
# heads of the current pair (2 x glen x 512 fp32): glen=2 -> 4 banks,
# glen=1 -> 2 banks; the two tiles double-buffer within 6 free banks and the
# two per-head attn accumulators take the other 2.
_JGROUPS = []
_j = 0
while _j < 32:
    g = 2 if (len(_JGROUPS) % 2 == 0 and _j + 2 <= 32) else 1
    _JGROUPS.append((_j, g))
    _j += g

import os
_STAGE = os.environ.get("KSTAGE", "full")  # debug bisection: proj|attn|nocc|full


def _emit(nc, tc, io, es):
    xs, src = io["xs"], io["src"]
    out = io["out"]

    wpool = es.enter_context(tc.tile_pool(name="weights", bufs=1))
    apool = es.enter_context(tc.tile_pool(name="acts", bufs=1))

    # ---------- weight / bias loads ----------
    wq_sb = wpool.tile([128, 2, D], FP)
    nc.sync.dma_start(out=wq_sb[:], in_=io["wqT"].rearrange("(c p) o -> p c o", p=128))
    wk_sb = wpool.tile([128, 2, D], FP)
    nc.sync.dma_start(out=wk_sb[:], in_=io["wkT"].rearrange("(c p) o -> p c o", p=128))
    wv_sb = wpool.tile([128, 2, D], FP)
    nc.sync.dma_start(out=wv_sb[:], in_=io["wvT"].rearrange("(c p) o -> p c o", p=128))
    wm_sb = wpool.tile([128, 2, D], BF)
    nc.gpsimd.dma_start(out=wm_sb[:], in_=io["wmT"].rearrange("(c p) o -> p c o", p=128))
    w1x_sb = wpool.tile([128, 2, 2 * D], FP)
    nc.sync.dma_start(out=w1x_sb[:], in_=io["w1xT"].rearrange("(c p) o -> p c o", p=128))
    w1m_sb = wpool.tile([128, 2, 2 * D], BF)
    nc.gpsimd.dma_start(out=w1m_sb[:], in_=io["w1mT"].rearrange("(c p) o -> p c o", p=128))
    w2_sb = wpool.tile([128, 4, D], BF)
    nc.gpsimd.dma_start(out=w2_sb[:], in_=io["w2T"].rearrange("(c p) o -> p c o", p=128))

    bq_sb = wpool.tile([128, 2], FP)
    nc.sync.dma_start(out=bq_sb[:], in_=io["bq"][:])
    bk_sb = wpool.tile([128, 2], FP)
    nc.sync.dma_start(out=bk_sb[:], in_=io["bk"][:])
    bm_sb = wpool.tile([128, 2], FP)
    nc.sync.dma_start(out=bm_sb[:], in_=io["bm"][:])
    b1_sb = wpool.tile([128, 4], FP)
    nc.sync.dma_start(out=b1_sb[:], in_=io["b1"][:])
    b2_sb = wpool.tile([128, 2], FP)
    nc.sync.dma_start(out=b2_sb[:], in_=io["b2"][:])
    bv_sb = wpool.tile([1, D], FP)
    nc.sync.dma_start(out=bv_sb[:], in_=io["bv"][:])
    bvb_sb = wpool.tile([128, D], FP)
    nc.gpsimd.partition_broadcast(bvb_sb[:], bv_sb[:])

    xs_sb = apool.tile([128, 2, NS], FP)
    nc.sync.dma_start(out=xs_sb[:], in_=xs.rearrange("(c p) n -> p c n", p=128))

    # ---------- persistent activation tiles ----------
    q_sb = apool.tile([128, 2, NS], BF)
    k_sb = apool.tile([128, 2, N], BF)
    # v^T per head + ones col, fp8, padded to stride 80 for DoubleRow
    vaT_sb = apool.tile([128, H, 16, 2, 80], F8)
    exp_sb = apool.tile([128, 2, 32, 512], F8)   # [., head-of-pair, m-chunk, n]
    attn_sb = apool.tile([128, 2, NS], BF)
    msg_sb = apool.tile([128, 2, NS], BF)
    h1_sb = apool.tile([128, 4, NS], FP)
    h1n_sb = apool.tile([128, 4, NS], BF)
    out_sb = apool.tile([128, 2, NS], FP)
    stats_sb = apool.tile([128, 8], FP)

    nc.vector.memset(vaT_sb[:, :, :, :, DH:DH + 1], 1.0)

    # ---------- phase 1: projections ----------
    with tc.tile_pool(name="srcp", bufs=1) as srcpool, \
         tc.tile_pool(name="pj", bufs=4, space="PSUM") as pj, \
         tc.tile_pool(name="vt", bufs=3, space="PSUM") as vtp:
        src_sb = srcpool.tile([128, 2, N], FP)
        nc.sync.dma_start(out=src_sb[:], in_=src.rearrange("(c p) m -> p c m", p=128))

        # q = WqT.T @ xs + bq   [256, NS]
        for oc in range(2):
            for ns in range(NS // 512):
                q_ps = pj.tile([128, 512], FP, tag="pj")
                for ic in range(2):
                    nc.tensor.matmul(
                        q_ps[:],
                        wq_sb[:, ic, oc * 128:(oc + 1) * 128],
                        xs_sb[:, ic, ns * 512:(ns + 1) * 512],
                        start=(ic == 0), stop=(ic == 1),
                    )
                nc.vector.tensor_scalar_add(
                    q_sb[:, oc, ns * 512:(ns + 1) * 512], q_ps[:], bq_sb[:, oc:oc + 1])

        # k = WkT.T @ src + bk   [256, N]
        for oc in range(2):
            for ns in range(N // 512):
                k_ps = pj.tile([128, 512], FP, tag="pj")
                for ic in range(2):
                    nc.tensor.matmul(
                        k_ps[:],
                        wk_sb[:, ic, oc * 128:(oc + 1) * 128],
                        src_sb[:, ic, ns * 512:(ns + 1) * 512],
                        start=(ic == 0), stop=(ic == 1),
                    )
                nc.vector.tensor_scalar_add(
                    k_sb[:, oc, ns * 512:(ns + 1) * 512], k_ps[:], bk_sb[:, oc:oc + 1])

        # v^T (+bias) directly transposed: out[m, c] = sum_i src[i, m] WvT[i, c]
        for mc in range(N // 128):
            vt_ps = vtp.tile([128, D], FP, tag="vt")
            for ic in range(2):
                nc.tensor.matmul(
                    vt_ps[:],
                    src_sb[:, ic, mc * 128:(mc + 1) * 128],
                    wv_sb[:, ic, :],
                    start=(ic == 0), stop=(ic == 1),
                )
            for h in range(H):
                nc.vector.tensor_add(
                    vaT_sb[:, h, mc // 2, mc % 2, 0:DH],
                    vt_ps[:, h * DH:(h + 1) * DH],
                    bvb_sb[:, h * DH:(h + 1) * DH],
                )

    if _STAGE == "proj":
        nc.vector.tensor_copy(out_sb[:], q_sb[:])
        nc.sync.dma_start(out=out.rearrange("(c p) n -> p c n", p=128), in_=out_sb[:])
        return

    # ---------- phase 2: attention ----------
    with tc.tile_pool(name="scA", bufs=1, space="PSUM") as scA, \
         tc.tile_pool(name="scB", bufs=1, space="PSUM") as scB, \
         tc.tile_pool(name="at", bufs=1, space="PSUM") as atp, \
         tc.tile_pool(name="nrm", bufs=4) as nrm:
        for hp in range(2):
            kc = hp
            for nch in range(NS // 512):
                n0 = nch * 512
                # scores_T[m, n] = k_h[:, m].T @ q_h[:, n] ; exp on ScalarE.
                # The pair's heads sit at base partitions 0/64, so adjacent
                # matmuls target disjoint PE row groups and run concurrently.
                for (j0, glen) in _JGROUPS:
                    pool = scA if glen == 2 else scB
                    sc_ps = pool.tile([128, 2, glen, 512], FP, tag=pool.name)
                    for j4 in range(glen):
                        j = j0 + j4
                        for hh in range(2):
                            bp = 64 * hh
                            nc.tensor.matmul(
                                sc_ps[:, hh, j4, :],
                                k_sb[bp:bp + DH, kc, j * 128:(j + 1) * 128],
                                q_sb[bp:bp + DH, kc, n0:n0 + 512],
                                start=True, stop=True,
                            )
                    nc.scalar.activation(
                        out=exp_sb[:, :, j0:j0 + glen, :], in_=sc_ps[:],
                        func=AF.Exp, scale=0.125)
                # attn (+Z) accumulate: out[0:64]=sum_m vT*exp, out[64]=sum_m exp
                # fp8 DoubleRow: two 128-row m-chunks per matmul pass
                for hh in range(2):
                    h, bp = 2 * hp + hh, 64 * hh
                    at_ps = atp.tile([128, 512], FP, tag=f"at{hh}")
                    for p in range(16):
                        nc.tensor.matmul(
                            at_ps[:DH + 1, :],
                            vaT_sb[:, h, p, :, 0:DH + 1],
                            exp_sb[:, hh, 2 * p:2 * p + 2, :],
                            start=(p == 0), stop=(p == 15),
                            perf_mode=mybir.MatmulPerfMode.DoubleRow,
                        )
                    rz = nrm.tile([1, 512], FP, tag="rz")
                    nc.vector.reciprocal(rz[:], at_ps[DH:DH + 1, :])
                    rzb = nrm.tile([DH, 512], FP, tag="rzb")
                    nc.gpsimd.partition_broadcast(rzb[:], rz[:])
                    nc.vector.tensor_mul(
                        attn_sb[bp:bp + DH, kc, n0:n0 + 512], at_ps[0:DH, :], rzb[:])

    if _STAGE == "attn":
        nc.vector.tensor_copy(out_sb[:], attn_sb[:])
        nc.sync.dma_start(out=out.rearrange("(c p) n -> p c n", p=128), in_=out_sb[:])
        return

    # ---------- phase 3: message, MLP, instance norm, output ----------
    with tc.tile_pool(name="mm", bufs=6, space="PSUM") as mm, \
         tc.tile_pool(name="dram", bufs=1, space="DRAM") as dram, \
         tc.tile_pool(name="nstat", bufs=1) as nstat:
        # message = WmT.T @ attn + bm
        for oc in range(2):
            for ns in range(NS // 512):
                m_ps = mm.tile([128, 512], FP, tag="mm")
                for ic in range(2):
                    nc.tensor.matmul(
                        m_ps[:],
                        wm_sb[:, ic, oc * 128:(oc + 1) * 128],
                        attn_sb[:, ic, ns * 512:(ns + 1) * 512],
                        start=(ic == 0), stop=(ic == 1),
                    )
                nc.vector.tensor_scalar_add(
                    msg_sb[:, oc, ns * 512:(ns + 1) * 512], m_ps[:], bm_sb[:, oc:oc + 1])

        if _STAGE == "msg":
            nc.vector.tensor_copy(out_sb[:], msg_sb[:])
            nc.sync.dma_start(out=out.rearrange("(c p) n -> p c n", p=128), in_=out_sb[:])
            return

        # h1 = W1T.T @ [xs; msg] + b1   [512, NS]
        for oc in range(4):
            for ns in range(NS // 512):
                h_ps = mm.tile([128, 512], FP, tag="mm")
                for ic in range(2):
                    nc.tensor.matmul(
                        h_ps[:],
                        w1x_sb[:, ic, oc * 128:(oc + 1) * 128],
                        xs_sb[:, ic, ns * 512:(ns + 1) * 512],
                        start=(ic == 0), stop=False,
                    )
                for ic in range(2):
                    nc.tensor.matmul(
                        h_ps[:],
                        w1m_sb[:, ic, oc * 128:(oc + 1) * 128],
                        msg_sb[:, ic, ns * 512:(ns + 1) * 512],
                        start=False, stop=(ic == 1),
                    )
                nc.vector.tensor_scalar_add(
                    h1_sb[:, oc, ns * 512:(ns + 1) * 512], h_ps[:], b1_sb[:, oc:oc + 1])

        if _STAGE == "h1":
            nc.vector.tensor_copy(out_sb[:, 0, :], h1_sb[:, 0, :])
            nc.vector.tensor_copy(out_sb[:, 1, :], h1_sb[:, 1, :])
            nc.sync.dma_start(out=out.rearrange("(c p) n -> p c n", p=128), in_=out_sb[:])
            return

        # per-core partial stats (sum, sumsq) over the local NS columns,
        # via bn_stats/bn_aggr (mean, biased var) -> scaled to (sum, sumsq)
        for t in range(4):
            bst = nstat.tile([128, 2, 6], FP, tag="bst")
            for g in range(2):
                nc.vector.bn_stats(out=bst[:, g, :], in_=h1_sb[:, t, g * 512:(g + 1) * 512])
            mv = nstat.tile([128, 2], FP, tag="mv")
            nc.vector.bn_aggr(out=mv[:], in_=bst[:])
            nc.vector.tensor_scalar_mul(stats_sb[:, t:t + 1], mv[:, 0:1], float(NS))
            msq = nstat.tile([128, 1], FP, tag="msq")
            nc.vector.tensor_mul(msq[:], mv[:, 0:1], mv[:, 0:1])
            msq2 = nstat.tile([128, 1], FP, tag="msq2")
            nc.vector.tensor_add(msq2[:], mv[:, 1:2], msq[:])
            nc.vector.tensor_scalar_mul(stats_sb[:, 4 + t:5 + t], msq2[:], float(NS))

        if _STAGE == "stats":
            nc.vector.tensor_copy(out_sb[:, 0, :], h1_sb[:, 0, :])
            nc.vector.tensor_copy(out_sb[:, 1, 0:8], stats_sb[:])
            nc.sync.dma_start(out=out.rearrange("(c p) n -> p c n", p=128), in_=out_sb[:])
            return

        # cross-core reduce within each batch group of 4 cores
        sred = nstat.tile([128, 8], FP)
        if _STAGE == "nocc":
            nc.vector.tensor_scalar_mul(sred[:], stats_sb[:], 4.0)
        else:
            cc_in = dram.tile([128, 8], FP)
            cc_out = dram.tile([128, 8], FP)
            nc.sync.dma_start(out=cc_in[:], in_=stats_sb[:])
            nc.gpsimd.collective_compute(
                "AllReduce", OP.add,
                replica_groups=[[0, 1, 2, 3], [4, 5, 6, 7]],
                ins=[cc_in[:].opt()], outs=[cc_out[:].opt()],
            )
            nc.sync.dma_start(out=sred[:], in_=cc_out[:])

        mu4 = nstat.tile([128, 4], FP)
        nc.vector.tensor_scalar_mul(mu4[:], sred[:, 0:4], 1.0 / N)
        e24 = nstat.tile([128, 4], FP)
        nc.vector.tensor_scalar_mul(e24[:], sred[:, 4:8], 1.0 / N)
        var4 = nstat.tile([128, 4], FP)
        nc.vector.tensor_mul(var4[:], mu4[:], mu4[:])
        nc.vector.tensor_tensor(out=var4[:], in0=e24[:], in1=var4[:], op=OP.subtract)
        eps1 = nstat.tile([128, 1], FP)
        nc.vector.memset(eps1[:], EPS)
        std4 = nstat.tile([128, 4], FP)
        nc.scalar.activation(out=std4[:], in_=var4[:], func=AF.Sqrt, bias=eps1[:])
        rstd4 = nstat.tile([128, 4], FP)
        nc.vector.reciprocal(rstd4[:], std4[:])
        nb4 = nstat.tile([128, 4], FP)
        nc.vector.tensor_mul(nb4[:], mu4[:], rstd4[:])
        nc.vector.tensor_scalar_mul(nb4[:], nb4[:], -1.0)

        # h = relu((h1 - mu) * rstd) = relu(h1 * rstd - mu * rstd)
        for t in range(4):
            nc.scalar.activation(
                out=h1n_sb[:, t, :], in_=h1_sb[:, t, :], func=AF.Relu,
                bias=nb4[:, t:t + 1], scale=rstd4[:, t:t + 1])

        # out = W2T.T @ h + b2
        for oc in range(2):
            for ns in range(NS // 512):
                o_ps = mm.tile([128, 512], FP, tag="mm")
                for kc2 in range(4):
                    nc.tensor.matmul(
                        o_ps[:],
                        w2_sb[:, kc2, oc * 128:(oc + 1) * 128],
                        h1n_sb[:, kc2, ns * 512:(ns + 1) * 512],
                        start=(kc2 == 0), stop=(kc2 == 3),
                    )
                nc.vector.tensor_scalar_add(
                    out_sb[:, oc, ns * 512:(ns + 1) * 512], o_ps[:], b2_sb[:, oc:oc + 1])

        nc.sync.dma_start(out=out.rearrange("(c p) n -> p c n", p=128), in_=out_sb[:])


_BUILT = {}


def _build():
    if "nc" in _BUILT:
        return _BUILT["nc"]
    nc = bacc.Bacc("TRN2", target_bir_lowering=False, debug=False,
                   enable_asserts=True, num_devices=NCORES)
    io = {}
    io["xs"] = nc.dram_tensor("xs", [D, NS], FP, kind="ExternalInput").ap()
    io["src"] = nc.dram_tensor("src", [D, N], FP, kind="ExternalInput").ap()
    io["wqT"] = nc.dram_tensor("wqT", [D, D], FP, kind="ExternalInput").ap()
    io["wkT"] = nc.dram_tensor("wkT", [D, D], FP, kind="ExternalInput").ap()
    io["wvT"] = nc.dram_tensor("wvT", [D, D], FP, kind="ExternalInput").ap()
    io["wmT"] = nc.dram_tensor("wmT", [D, D], FP, kind="ExternalInput").ap()
    io["w1xT"] = nc.dram_tensor("w1xT", [D, 2 * D], FP, kind="ExternalInput").ap()
    io["w1mT"] = nc.dram_tensor("w1mT", [D, 2 * D], FP, kind="ExternalInput").ap()
    io["w2T"] = nc.dram_tensor("w2T", [2 * D, D], FP, kind="ExternalInput").ap()
    io["bq"] = nc.dram_tensor("bq", [128, 2], FP, kind="ExternalInput").ap()
    io["bk"] = nc.dram_tensor("bk", [128, 2], FP, kind="ExternalInput").ap()
    io["bv"] = nc.dram_tensor("bv", [1, D], FP, kind="ExternalInput").ap()
    io["bm"] = nc.dram_tensor("bm", [128, 2], FP, kind="ExternalInput").ap()
    io["b1"] = nc.dram_tensor("b1", [128, 4], FP, kind="ExternalInput").ap()
    io["b2"] = nc.dram_tensor("b2", [128, 2], FP, kind="ExternalInput").ap()
    io["out"] = nc.dram_tensor("out", [D, NS], FP, kind="ExternalOutput").ap()

    import contextlib
    with tile.TileContext(nc) as tc:
        with contextlib.ExitStack() as es:
            _emit(nc, tc, io, es)
    nc.compile()
    _BUILT["nc"] = nc
    return nc


def _prep_inputs(x, source, Wq, bq, Wk, bk, Wv, bv, Wm, bm, W1, b1, W2, b2):
    perm = np.array([4 * d + h for h in range(H) for d in range(DH)])
    f32 = lambda a: np.ascontiguousarray(a, dtype=np.float32)

    shared = {
        "wqT": f32(Wq[perm, :].T),
        "wkT": f32(Wk[perm, :].T),
        "wvT": f32(Wv[perm, :].T),
        "wmT": f32(Wm[:, perm].T),
        "w1xT": f32(W1.T[0:D, :]),
        "w1mT": f32(W1.T[D:2 * D, :]),
        "w2T": f32(W2.T),
        "bq": f32(bq[perm].reshape(2, 128).T),
        "bk": f32(bk[perm].reshape(2, 128).T),
        "bv": f32(bv[perm].reshape(1, D)),
        "bm": f32(bm.reshape(2, 128).T),
        "b1": f32(b1.reshape(4, 128).T),
        "b2": f32(b2.reshape(2, 128).T),
    }
    in_maps = []
    for core in range(NCORES):
        b, s = core // 4, core % 4
        m = dict(shared)
        m["xs"] = f32(x[b][:, s * NS:(s + 1) * NS])
        m["src"] = f32(source[b])
        in_maps.append(m)
    return in_maps


def run(inputs, **spmd_kwargs):
    """Build (cached), run on cores 0-7, return (full_output, BassKernelResults)."""
    nc = _build()
    in_maps = _prep_inputs(**inputs)
    res = bass_utils.run_bass_kernel_spmd(
        nc, in_maps, core_ids=list(range(NCORES)), **spmd_kwargs)
    full = np.empty((B, D, N), dtype=np.float32)
    for core in range(NCORES):
        b, s = core // 4, core % 4
        full[b][:, s * NS:(s + 1) * NS] = res.results[core]["out"]
    return full, res


def kernel(**inputs):
    full, _ = run(inputs)
    return full



# revision 3
# speedup vs baseline: 1.4767x; 1.4767x over previous
# Trainium2 Bass kernel for nn_AttentionalPropagation (B=2, D=256, N=M=4096, H=4).
#
# Sharding: 8 cores; each batch (B=2) owns 4 cores; each core computes a
# 1024-column sequence shard of the output end-to-end. k,v are computed
# redundantly per core from the full `source` of its batch. The only
# cross-core communication is an AllGather of InstanceNorm partial
# (sum, sumsq) stats within each 4-core batch group (summed locally).
#
# Schedule: the ScalarE exp stream over the 4096x4096 score matrix is the
# structural bottleneck (~125us); everything else (projections in bf16,
# attn fp8-DoubleRow accumulation, message/MLP matmuls, stats) is emitted
# as "filler" interleaved into the exp-paced phase loop so PE/DVE run in
# the shadow of ACT. Scores double-buffer through PSUM (4+2 banks) while
# the two attn accumulators hold 2 banks; per-phase mm/at pools time-share
# the remaining space for projections and the MLP.
#
# Per-head layout trick (as baseline): conv weights' output channels are
# permuted host-side so head h lives at partitions 64*(h%2) of channel
# chunk h//2, letting per-head matmuls run off partition-aligned slices.
#
# Softmax: scores are built transposed ([m, n], m on partitions), exp'd on
# ScalarE (scale=1/8 folded; |s/8| < ~5 so no max subtraction), written as
# fp8 into a parity-double-buffered SBUF tile, and the denominator comes
# free as row 64 of the attn matmul via a ones column appended to v^T.

import numpy as np
import ml_dtypes

import concourse.bass as bass  # noqa: F401
import concourse.tile as tile
import concourse.mybir as mybir
from concourse import bacc
from concourse import bass_utils

B, D, N = 2, 256, 4096
H, DH = 4, 64
NS = N // 4           # sequence shard per core
NCORES = 8
EPS = 1e-5
NBLK = 2              # 512-col blocks per core
W0 = 512

FP = mybir.dt.float32
BF = mybir.dt.bfloat16
F8 = mybir.dt.float8e4
OP = mybir.AluOpType
AF = mybir.ActivationFunctionType
DR = mybir.MatmulPerfMode.DoubleRow

# j-groups for the scores->exp pipeline: glen-2 groups (4-bank PSUM tile)
# alternate with glen-1 groups (2 banks) so two tiles double-buffer in 6
# banks while the attn accumulators keep the other 2.
JGROUPS = []
_j = 0
while _j < 32:
    g = 2 if (len(JGROUPS) % 2 == 0 and _j + 2 <= 32) else 1
    JGROUPS.append((_j, g))
    _j += g

import os
_STAGE = os.environ.get("KSTAGE", "full")


def _emit(nc, tc, io, es):
    wpool = es.enter_context(tc.tile_pool(name="weights", bufs=1))
    apool = es.enter_context(tc.tile_pool(name="acts", bufs=1))
    npool = es.enter_context(tc.tile_pool(name="small", bufs=4))

    # ---------- persistent tiles ----------
    wq_sb = wpool.tile([128, 2, D], BF)
    wk_sb = wpool.tile([128, 2, D], BF)
    wv_sb = wpool.tile([128, 2, D], BF)
    wm_sb = wpool.tile([128, 2, D], BF)
    w1x_sb = wpool.tile([128, 2, 2 * D], BF)
    w1m_sb = wpool.tile([128, 2, 2 * D], BF)
    w2_sb = wpool.tile([128, 4, D], BF)
    bq_sb = wpool.tile([128, 2], FP)
    bk_sb = wpool.tile([128, 2], FP)
    bm_sb = wpool.tile([128, 2], FP)
    b1_sb = wpool.tile([128, 4], FP)
    b2_sb = wpool.tile([128, 2], FP)
    bv_sb = wpool.tile([1, D], BF)
    ones_sb = wpool.tile([1, 128], BF)

    xs_sb = apool.tile([128, 2, NS], BF)
    src_sb = apool.tile([128, 2, N], BF)
    q_sb = apool.tile([128, 2, NS], BF)
    k_sb = apool.tile([128, 2, N], BF)
    # v^T per m-pair-chunk: [pair-chunk, ko, head, 80]; col DH is the ones
    # column (softmax denominator); stride 80 keeps DoubleRow's step%16==0.
    vaT_sb = apool.tile([128, 16, 2, H, 80], F8)
    # exp double-buffered by phase parity: [parity, head-of-pair, m-chunk, n]
    exp_sb = apool.tile([128, 2, 2, 32, W0], F8)
    attn_sb = apool.tile([128, 2, NS], BF)
    msg_sb = apool.tile([128, 2, NS], BF)
    h1_sb = apool.tile([128, 4, NS], FP)
    h1n_sb = apool.tile([128, 4, NS], BF)
    out_sb = apool.tile([128, 2, NS], FP)
    bst_sb = apool.tile([128, 4, NBLK, 6], FP)
    stats_sb = apool.tile([128, 8], FP)

    # ---------- input DMAs, in need-order ----------
    xs, src = io["xs"], io["src"]
    nc.sync.dma_start(out=xs_sb[:], in_=xs.rearrange("(c p) n -> p c n", p=128))
    nc.sync.dma_start(out=wq_sb[:], in_=io["wqT"].rearrange("(c p) o -> p c o", p=128))
    nc.sync.dma_start(out=bq_sb[:], in_=io["bq"][:])
    srcr = src.rearrange("(c p) m -> p c m", p=128)
    nc.sync.dma_start(out=src_sb[:, :, 0:1024], in_=srcr[:, :, 0:1024])
    nc.sync.dma_start(out=wk_sb[:], in_=io["wkT"].rearrange("(c p) o -> p c o", p=128))
    nc.sync.dma_start(out=bk_sb[:], in_=io["bk"][:])
    nc.sync.dma_start(out=wv_sb[:], in_=io["wvT"].rearrange("(c p) o -> p c o", p=128))
    nc.sync.dma_start(out=bv_sb[:], in_=io["bv"][:])
    for sc in range(1, 4):
        nc.sync.dma_start(out=src_sb[:, :, 1024 * sc:1024 * (sc + 1)],
                          in_=srcr[:, :, 1024 * sc:1024 * (sc + 1)])
    nc.sync.dma_start(out=wm_sb[:], in_=io["wmT"].rearrange("(c p) o -> p c o", p=128))
    nc.sync.dma_start(out=bm_sb[:], in_=io["bm"][:])
    nc.sync.dma_start(out=w1x_sb[:], in_=io["w1xT"].rearrange("(c p) o -> p c o", p=128))
    nc.sync.dma_start(out=w1m_sb[:], in_=io["w1mT"].rearrange("(c p) o -> p c o", p=128))
    nc.sync.dma_start(out=b1_sb[:], in_=io["b1"][:])
    nc.sync.dma_start(out=w2_sb[:], in_=io["w2T"].rearrange("(c p) o -> p c o", p=128))
    nc.sync.dma_start(out=b2_sb[:], in_=io["b2"][:])

    nc.vector.memset(vaT_sb[:, :, :, :, DH:DH + 1], 1.0)
    nc.vector.memset(ones_sb[:], 1.0)

    # ---------- filler quanta (PE work interleaved into the phase loop) ----
    def f_q(oc, nsb, mm):
        ps = mm.tile([128, 512], FP, tag="mm")
        for ic in range(2):
            nc.tensor.matmul(ps[:], wq_sb[:, ic, oc * 128:(oc + 1) * 128],
                             xs_sb[:, ic, nsb * 512:(nsb + 1) * 512],
                             start=(ic == 0), stop=(ic == 1))
        nc.vector.tensor_scalar_add(
            q_sb[:, oc, nsb * 512:(nsb + 1) * 512], ps[:], bq_sb[:, oc:oc + 1])

    def f_k(oc, half, mm):
        ps = mm.tile([128, 512], FP, tag="mm")
        for ic in range(2):
            nc.tensor.matmul(ps[:], wk_sb[:, ic, oc * 128:(oc + 1) * 128],
                             src_sb[:, ic, half * 512:(half + 1) * 512],
                             start=(ic == 0), stop=(ic == 1))
        nc.vector.tensor_scalar_add(
            k_sb[:, oc, half * 512:(half + 1) * 512], ps[:], bk_sb[:, oc:oc + 1])

    def f_v(pair, mm):
        # v^T (+bias via ones row) for m columns [256*pair, 256*pair+256)
        ps = mm.tile([128, 2, H, DH], FP, tag="mm")
        for par in range(2):
            mc = 2 * pair + par
            for ic in range(2):
                nc.tensor.matmul(ps[:, par], src_sb[:, ic, mc * 128:(mc + 1) * 128],
                                 wv_sb[:, ic, :], start=(ic == 0), stop=False)
            nc.tensor.matmul(ps[:, par], ones_sb[:, 0:128],
                             bv_sb[:], start=False, stop=True)
        nc.vector.tensor_copy(vaT_sb[:, pair, :, :, 0:DH], ps[:])

    def f_msg(b, mm):
        n0 = b * W0
        for oc in range(2):
            ps = mm.tile([128, 512], FP, tag="mm")
            for ic in range(2):
                nc.tensor.matmul(ps[:], wm_sb[:, ic, oc * 128:(oc + 1) * 128],
                                 attn_sb[:, ic, n0:n0 + W0],
                                 start=(ic == 0), stop=(ic == 1))
            nc.vector.tensor_scalar_add(
                msg_sb[:, oc, n0:n0 + W0], ps[:], bm_sb[:, oc:oc + 1])

    def f_h1(b, t, mm):
        n0 = b * W0
        ps = mm.tile([128, 512], FP, tag="mm")
        for ic in range(2):
            nc.tensor.matmul(ps[:], w1x_sb[:, ic, t * 128:(t + 1) * 128],
                             xs_sb[:, ic, n0:n0 + W0], start=(ic == 0), stop=False)
        for ic in range(2):
            nc.tensor.matmul(ps[:], w1m_sb[:, ic, t * 128:(t + 1) * 128],
                             msg_sb[:, ic, n0:n0 + W0], start=False, stop=(ic == 1))
        nc.vector.tensor_scalar_add(h1_sb[:, t, n0:n0 + W0], ps[:], b1_sb[:, t:t + 1])
        nc.vector.bn_stats(out=bst_sb[:, t, b, :], in_=h1_sb[:, t, n0:n0 + W0])

    # ---------- scores / exp / attn emitters ----------
    def emit_group(b, hp, parity, gi, scA, scB):
        j0, glen = JGROUPS[gi]
        n0 = b * W0
        pool, tag = (scA, "scA") if glen == 2 else (scB, "scB")
        sc_ps = pool.tile([128, 2, glen, W0], FP, tag=tag)
        for j4 in range(glen):
            jj = j0 + j4
            for hh in range(2):
                bp = 64 * hh
                nc.tensor.matmul(
                    sc_ps[:, hh, j4, :],
                    k_sb[bp:bp + DH, hp, jj * 128:(jj + 1) * 128],
                    q_sb[bp:bp + DH, hp, n0:n0 + W0],
                    start=True, stop=True)
        nc.scalar.activation(
            out=exp_sb[:, parity, :, j0:j0 + glen, :], in_=sc_ps[:],
            func=AF.Exp, scale=0.125)

    def emit_attn(b, hp, parity, atp):
        n0 = b * W0
        for hh in range(2):
            h, bp = 2 * hp + hh, 64 * hh
            at_ps = atp.tile([128, W0], FP, tag=f"at{hh}")
            for p in range(16):
                nc.tensor.matmul(
                    at_ps[:DH + 1, :],
                    vaT_sb[:, p, :, h, 0:DH + 1],
                    exp_sb[:, parity, hh, 2 * p:2 * p + 2, :],
                    start=(p == 0), stop=(p == 15),
                    perf_mode=DR)
            rz = npool.tile([1, W0], FP, tag="rz")
            nc.vector.reciprocal(rz[:], at_ps[DH:DH + 1, :])
            rzb = npool.tile([DH, W0], FP, tag="rzb")
            nc.gpsimd.partition_broadcast(rzb[:], rz[:])
            nc.vector.tensor_mul(
                attn_sb[bp:bp + DH, hp, n0:n0 + W0], at_ps[0:DH, :], rzb[:])

    # ---------- pre-phase: q oc0 + k oc0 first chunk ----------
    with tc.tile_pool(name="pre", bufs=2, space="PSUM") as pre:
        f_q(0, 0, pre)
        f_q(0, 1, pre)
        f_k(0, 0, pre)
        f_k(0, 1, pre)

    if _STAGE == "proj":
        # also run the rest of the projections, then dump q
        with tc.tile_pool(name="pp", bufs=2, space="PSUM") as pp:
            f_q(1, 0, pp)
            f_q(1, 1, pp)
            for hf in range(2, 8):
                f_k(0, hf, pp)
            for hf in range(8):
                f_k(1, hf, pp)
            for pair in range(16):
                f_v(pair, pp)
        nc.vector.tensor_copy(out_sb[:], q_sb[:])
        nc.sync.dma_start(out=io["out"].rearrange("(c p) n -> p c n", p=128),
                          in_=out_sb[:])
        return

    # ---------- filler schedule ----------
    # (need-by order: k oc0 halves 2..7 feed phase-0 scores; q oc1 + k oc1
    # first halves feed phase 1; v pairs feed the attn passes that run at the
    # END of the phase after their exp; mlp(b) runs in phase (b+1,hp1)'s
    # window or the tail.)
    fillers = []
    fillers += [lambda mm, hf=hf: f_k(0, hf, mm) for hf in (2, 3)]
    fillers += [lambda mm, oc=oc: f_q(1, oc, mm) for oc in (0, 1)]
    fillers += [lambda mm, hf=hf: f_k(1, hf, mm) for hf in (0, 1)]
    fillers += [lambda mm, hf=hf: f_k(0, hf, mm) for hf in (4, 5)]
    fillers += [lambda mm, p=p: f_v(p, mm) for p in (0, 1, 2)]
    fillers += [lambda mm, hf=hf: f_k(0, hf, mm) for hf in (6, 7)]
    fillers += [lambda mm, p=p: f_v(p, mm) for p in (3, 4, 5)]
    fillers += [lambda mm, hf=hf: f_k(1, hf, mm) for hf in (2, 3)]
    fillers += [lambda mm, p=p: f_v(p, mm) for p in (6, 7, 8)]
    fillers += [lambda mm, hf=hf: f_k(1, hf, mm) for hf in (4, 5)]
    fillers += [lambda mm, p=p: f_v(p, mm) for p in (9, 10, 11)]
    fillers += [lambda mm, hf=hf: f_k(1, hf, mm) for hf in (6, 7)]
    fillers += [lambda mm, p=p: f_v(p, mm) for p in (12, 13, 14, 15)]

    def mlp_block(b):
        return ([lambda mm, b=b: f_msg(b, mm)] +
                [lambda mm, b=b, t=t: f_h1(b, t, mm) for t in range(4)])

    phases = [(b, hp) for b in range(NBLK) for hp in range(2)]
    phase_extra = {3: mlp_block(0)}  # mlp(b0) after norm(b0,hp1) in phase 2

    with tc.tile_pool(name="scA", bufs=1, space="PSUM") as scA, \
         tc.tile_pool(name="scB", bufs=1, space="PSUM") as scB:
        for pi, (b, hp) in enumerate(phases):
            parity = pi % 2
            todo = list(phase_extra.get(pi, []))
            # prime the scores double-buffer
            emit_group(b, hp, parity, 0, scA, scB)
            emit_group(b, hp, parity, 1, scA, scB)
            with tc.tile_pool(name="mm", bufs=2, space="PSUM") as mm:
                for gi in range(2, len(JGROUPS)):
                    emit_group(b, hp, parity, gi, scA, scB)
                    take = todo if todo else fillers
                    if take:
                        take.pop(0)(mm)
                # drain leftovers for this phase
                for fn in todo:
                    fn(mm)
            # attn for the PREVIOUS phase at the end of this one
            if pi > 0:
                pb, php = phases[pi - 1]
                with tc.tile_pool(name="at", bufs=1, space="PSUM") as atp:
                    emit_attn(pb, php, (pi - 1) % 2, atp)

    # tail: attn of the last phase
    with tc.tile_pool(name="at", bufs=1, space="PSUM") as atp:
        emit_attn(NBLK - 1, 1, (len(phases) - 1) % 2, atp)

    if _STAGE == "attn":
        for fn in fillers:
            pass
        nc.vector.tensor_copy(out_sb[:], attn_sb[:])
        nc.sync.dma_start(out=io["out"].rearrange("(c p) n -> p c n", p=128),
                          in_=out_sb[:])
        return

    # tail: mlp of the last block
    with tc.tile_pool(name="mm", bufs=2, space="PSUM") as mm:
        for fn in mlp_block(NBLK - 1):
            fn(mm)

        if _STAGE == "h1":
            nc.vector.tensor_copy(out_sb[:, 0, :], h1_sb[:, 0, :])
            nc.vector.tensor_copy(out_sb[:, 1, :], h1_sb[:, 1, :])
            nc.sync.dma_start(out=io["out"].rearrange("(c p) n -> p c n", p=128),
                              in_=out_sb[:])
            return

        # ---------- stats: finalize, AllGather, combine ----------
        with tc.tile_pool(name="dram", bufs=1, space="DRAM") as dram:
            for t in range(4):
                mv = npool.tile([128, 2], FP, tag="mv")
                nc.vector.bn_aggr(out=mv[:], in_=bst_sb[:, t, :, :])
                nc.vector.tensor_scalar_mul(stats_sb[:, t:t + 1], mv[:, 0:1], float(NS))
                msq = npool.tile([128, 1], FP, tag="msq")
                nc.vector.tensor_mul(msq[:], mv[:, 0:1], mv[:, 0:1])
                msq2 = npool.tile([128, 1], FP, tag="msq2")
                nc.vector.tensor_add(msq2[:], mv[:, 1:2], msq[:])
                nc.vector.tensor_scalar_mul(stats_sb[:, 4 + t:5 + t], msq2[:], float(NS))

            sred = npool.tile([128, 4, 8], FP, tag="sred")
            if _STAGE == "nocc":
                for c in range(4):
                    nc.vector.tensor_copy(sred[:, c, :], stats_sb[:])
            else:
                cc_in = dram.tile([128, 8], FP)
                cc_out = dram.tile([4, 128, 8], FP)
                nc.sync.dma_start(out=cc_in[:], in_=stats_sb[:])
                nc.gpsimd.collective_compute(
                    "AllGather", OP.bypass,
                    replica_groups=[[0, 1, 2, 3], [4, 5, 6, 7]],
                    ins=[cc_in[:].opt()], outs=[cc_out[:].opt()],
                )
                nc.sync.dma_start(out=sred[:], in_=cc_out[:].rearrange("c p s -> p c s"))

            s01 = npool.tile([128, 8], FP, tag="s01")
            nc.vector.tensor_add(s01[:], sred[:, 0, :], sred[:, 1, :])
            s23 = npool.tile([128, 8], FP, tag="s23")
            nc.vector.tensor_add(s23[:], sred[:, 2, :], sred[:, 3, :])
            stot = npool.tile([128, 8], FP, tag="stot")
            nc.vector.tensor_add(stot[:], s01[:], s23[:])

            mu4 = npool.tile([128, 4], FP, tag="mu4")
            nc.vector.tensor_scalar_mul(mu4[:], stot[:, 0:4], 1.0 / N)
            e24 = npool.tile([128, 4], FP, tag="e24")
            nc.vector.tensor_scalar_mul(e24[:], stot[:, 4:8], 1.0 / N)
            var4 = npool.tile([128, 4], FP, tag="var4")
            nc.vector.tensor_mul(var4[:], mu4[:], mu4[:])
            nc.vector.tensor_tensor(out=var4[:], in0=e24[:], in1=var4[:], op=OP.subtract)
            eps1 = npool.tile([128, 1], FP, tag="eps1")
            nc.vector.memset(eps1[:], EPS)
            std4 = npool.tile([128, 4], FP, tag="std4")
            nc.scalar.activation(out=std4[:], in_=var4[:], func=AF.Sqrt, bias=eps1[:])
            rstd4 = npool.tile([128, 4], FP, tag="rstd4")
            nc.vector.reciprocal(rstd4[:], std4[:])
            nb4 = npool.tile([128, 4], FP, tag="nb4")
            nc.vector.tensor_mul(nb4[:], mu4[:], rstd4[:])
            nc.vector.tensor_scalar_mul(nb4[:], nb4[:], -1.0)

        # ---------- relu + W2 + out, pipelined per 512-col half ----------
        outr = io["out"].rearrange("(c p) n -> p c n", p=128)
        for nsb in range(2):
            for t in range(4):
                nc.scalar.activation(
                    out=h1n_sb[:, t, nsb * 512:(nsb + 1) * 512],
                    in_=h1_sb[:, t, nsb * 512:(nsb + 1) * 512], func=AF.Relu,
                    bias=nb4[:, t:t + 1], scale=rstd4[:, t:t + 1])
            for oc in range(2):
                o_ps = mm.tile([128, 512], FP, tag="mm")
                for kc2 in range(4):
                    nc.tensor.matmul(
                        o_ps[:],
                        w2_sb[:, kc2, oc * 128:(oc + 1) * 128],
                        h1n_sb[:, kc2, nsb * 512:(nsb + 1) * 512],
                        start=(kc2 == 0), stop=(kc2 == 3))
                nc.vector.tensor_scalar_add(
                    out_sb[:, oc, nsb * 512:(nsb + 1) * 512], o_ps[:],
                    b2_sb[:, oc:oc + 1])
            nc.sync.dma_start(out=outr[:, :, nsb * 512:(nsb + 1) * 512],
                              in_=out_sb[:, :, nsb * 512:(nsb + 1) * 512])


_BUILT = {}


def _build():
    if "nc" in _BUILT:
        return _BUILT["nc"]
    nc = bacc.Bacc("TRN2", target_bir_lowering=False, debug=False,
                   enable_asserts=True, num_devices=NCORES)
    io = {}
    io["xs"] = nc.dram_tensor("xs", [D, NS], BF, kind="ExternalInput").ap()
    io["src"] = nc.dram_tensor("src", [D, N], BF, kind="ExternalInput").ap()
    io["wqT"] = nc.dram_tensor("wqT", [D, D], BF, kind="ExternalInput").ap()
    io["wkT"] = nc.dram_tensor("wkT", [D, D], BF, kind="ExternalInput").ap()
    io["wvT"] = nc.dram_tensor("wvT", [D, D], BF, kind="ExternalInput").ap()
    io["wmT"] = nc.dram_tensor("wmT", [D, D], BF, kind="ExternalInput").ap()
    io["w1xT"] = nc.dram_tensor("w1xT", [D, 2 * D], BF, kind="ExternalInput").ap()
    io["w1mT"] = nc.dram_tensor("w1mT", [D, 2 * D], BF, kind="ExternalInput").ap()
    io["w2T"] = nc.dram_tensor("w2T", [2 * D, D], BF, kind="ExternalInput").ap()
    io["bq"] = nc.dram_tensor("bq", [128, 2], FP, kind="ExternalInput").ap()
    io["bk"] = nc.dram_tensor("bk", [128, 2], FP, kind="ExternalInput").ap()
    io["bv"] = nc.dram_tensor("bv", [1, D], BF, kind="ExternalInput").ap()
    io["bm"] = nc.dram_tensor("bm", [128, 2], FP, kind="ExternalInput").ap()
    io["b1"] = nc.dram_tensor("b1", [128, 4], FP, kind="ExternalInput").ap()
    io["b2"] = nc.dram_tensor("b2", [128, 2], FP, kind="ExternalInput").ap()
    io["out"] = nc.dram_tensor("out", [D, NS], FP, kind="ExternalOutput").ap()

    import contextlib
    with tile.TileContext(nc) as tc:
        with contextlib.ExitStack() as es:
            _emit(nc, tc, io, es)
    nc.compile()
    _BUILT["nc"] = nc
    return nc


def _prep_inputs(x, source, Wq, bq, Wk, bk, Wv, bv, Wm, bm, W1, b1, W2, b2):
    perm = np.array([4 * d + h for h in range(H) for d in range(DH)])
    f32 = lambda a: np.ascontiguousarray(a, dtype=np.float32)
    bf = lambda a: np.ascontiguousarray(np.asarray(a, dtype=np.float32),
                                        ).astype(ml_dtypes.bfloat16)

    shared = {
        "wqT": bf(Wq[perm, :].T),
        "wkT": bf(Wk[perm, :].T),
        "wvT": bf(Wv[perm, :].T),
        "wmT": bf(Wm[:, perm].T),
        "w1xT": bf(W1.T[0:D, :]),
        "w1mT": bf(W1.T[D:2 * D, :]),
        "w2T": bf(W2.T),
        "bq": f32(bq[perm].reshape(2, 128).T),
        "bk": f32(bk[perm].reshape(2, 128).T),
        "bv": bf(bv[perm].reshape(1, D)),
        "bm": f32(bm.reshape(2, 128).T),
        "b1": f32(b1.reshape(4, 128).T),
        "b2": f32(b2.reshape(2, 128).T),
    }
    in_maps = []
    for core in range(NCORES):
        b, s = core // 4, core % 4
        m = dict(shared)
        m["xs"] = bf(x[b][:, s * NS:(s + 1) * NS])
        m["src"] = bf(source[b])
        in_maps.append(m)
    return in_maps


def run(inputs, **spmd_kwargs):
    """Build (cached), run on cores 0-7, return (full_output, BassKernelResults)."""
    nc = _build()
    in_maps = _prep_inputs(**inputs)
    res = bass_utils.run_bass_kernel_spmd(
        nc, in_maps, core_ids=list(range(NCORES)), **spmd_kwargs)
    full = np.empty((B, D, N), dtype=np.float32)
    for core in range(NCORES):
        b, s = core // 4, core % 4
        full[b][:, s * NS:(s + 1) * NS] = res.results[core]["out"]
    return full, res


def kernel(**inputs):
    full, _ = run(inputs)
    return full


# revision 13
# speedup vs baseline: 1.5383x; 1.0417x over previous
# Trainium2 Bass kernel for nn_AttentionalPropagation (B=2, D=256, N=M=4096, H=4).
#
# Sharding: 8 cores; each batch (B=2) owns 4 cores; each core computes a
# 1024-column sequence shard of the output end-to-end. k,v are computed
# redundantly per core from the full `source` of its batch. The only
# cross-core communication is an AllGather of InstanceNorm partial
# (sum, sumsq) stats within each 4-core batch group (summed locally).
#
# Schedule: the ScalarE exp stream over the score matrix is the structural
# bottleneck (~125us per core); everything else (bf16 projections, fp8
# DoubleRow attn accumulation, message/MLP, stats) is interleaved into the
# exp-paced phase loop so PE/DVE run in ACT's shadow. Each phase (block,
# head-pair) streams 21 score j-groups through a 4+2-bank PSUM double
# buffer; the remaining 2 banks time-share between an "mm" window (matmul
# fillers: projections / MLP of earlier blocks) and an "at" window where
# the previous phase's attn accumulation interleaves with the last score
# groups. exp output is parity-double-buffered so attn of phase P never
# blocks exp of phase P+1.
#
# Per-head layout trick (as baseline): conv weights' output channels are
# permuted host-side so head h lives at partitions 64*(h%2) of channel
# chunk h//2, letting per-head matmuls run off partition-aligned slices.
#
# Softmax: scores are built transposed ([m, n], m on partitions), exp'd on
# ScalarE (scale=1/8 folded; |s/8| < ~5 so no max subtraction), written as
# fp8, and the denominator comes free as row 64 of the attn matmul via a
# ones column appended to v^T.

import numpy as np
import ml_dtypes

import concourse.bass as bass  # noqa: F401
import concourse.tile as tile
import concourse.mybir as mybir
from concourse import bacc
from concourse import bass_utils

B, D, N = 2, 256, 4096
H, DH = 4, 64
NS = N // 4           # sequence shard per core
NCORES = 8
EPS = 1e-5
NBLK = 2              # 512-col blocks per core
W0 = 512

FP = mybir.dt.float32
BF = mybir.dt.bfloat16
F8 = mybir.dt.float8e4
OP = mybir.AluOpType
AF = mybir.ActivationFunctionType
DR = mybir.MatmulPerfMode.DoubleRow

# j-groups for the scores->exp pipeline: glen-2 groups (4-bank PSUM tile)
# alternate with glen-1 groups (2 banks) so two tiles double-buffer in 6
# banks while the attn accumulators keep the other 2.
JGROUPS = []
_j = 0
while _j < 32:
    g = 2 if (len(JGROUPS) % 2 == 0 and _j + 2 <= 32) else 1
    JGROUPS.append((_j, g))
    _j += g
NG = len(JGROUPS)  # 21

import os
_STAGE = os.environ.get("KSTAGE", "full")


def _emit(nc, tc, io, es):
    wpool = es.enter_context(tc.tile_pool(name="weights", bufs=1))
    apool = es.enter_context(tc.tile_pool(name="acts", bufs=1))
    npool = es.enter_context(tc.tile_pool(name="small", bufs=4))

    # ---------- persistent tiles ----------
    wq_sb = wpool.tile([128, 2, D], BF)
    wk_sb = wpool.tile([128, 2, D], BF)
    wv_sb = wpool.tile([128, 2, D], BF)
    wm_sb = wpool.tile([128, 2, D], BF)
    w1x_sb = wpool.tile([128, 2, 2 * D], BF)
    w1m_sb = wpool.tile([128, 2, 2 * D], BF)
    w2_sb = wpool.tile([128, 4, D], BF)
    bq_sb = wpool.tile([128, 2], FP)
    bk_sb = wpool.tile([128, 2], FP)
    bm_sb = wpool.tile([128, 2], FP)
    b1_sb = wpool.tile([128, 4], FP)
    b2_sb = wpool.tile([128, 2], FP)
    bv_sb = wpool.tile([1, 2 * D], FP)       # v bias duplicated host-side per m-pair
    bvb_sb = wpool.tile([128, 2, H, DH], FP)

    xs_sb = apool.tile([128, 2, NS], BF)
    src_sb = apool.tile([128, 2, N], BF)
    q_sb = apool.tile([128, 2, NS], BF)
    k_sb = apool.tile([128, 2, N], BF)
    # v^T per m-pair-chunk: [pair-chunk, ko, head, 80]; col DH is the ones
    # column (softmax denominator); stride 80 keeps DoubleRow's step%16==0.
    vaT_sb = apool.tile([128, 16, 2, H, 80], F8)
    # exp double-buffered by phase parity: [parity, head-of-pair, m-chunk, n]
    exp_sb = apool.tile([128, 2, 2, 32, W0], F8)
    attn_sb = apool.tile([128, 2, NS], BF)
    msg_sb = apool.tile([128, 2, NS], BF)
    h1_sb = apool.tile([128, 4, NS], FP)
    h1n_sb = apool.tile([128, 4, NS], BF)
    out_sb = apool.tile([128, 2, NS], FP)
    bst_sb = apool.tile([128, 4, NBLK, 6], FP)
    stats_sb = apool.tile([128, 8], FP)

    # ---------- input DMAs, in need-order ----------
    xs, src = io["xs"], io["src"]
    nc.sync.dma_start(out=xs_sb[:], in_=xs.rearrange("(c p) n -> p c n", p=128))
    nc.sync.dma_start(out=wq_sb[:], in_=io["wqT"].rearrange("(c p) o -> p c o", p=128))
    nc.sync.dma_start(out=bq_sb[:], in_=io["bq"][:])
    srcr = src.rearrange("(c p) m -> p c m", p=128)
    nc.sync.dma_start(out=src_sb[:, :, 0:1024], in_=srcr[:, :, 0:1024])
    nc.sync.dma_start(out=wk_sb[:], in_=io["wkT"].rearrange("(c p) o -> p c o", p=128))
    nc.sync.dma_start(out=bk_sb[:], in_=io["bk"][:])
    nc.sync.dma_start(out=wv_sb[:], in_=io["wvT"].rearrange("(c p) o -> p c o", p=128))
    nc.sync.dma_start(out=bv_sb[:], in_=io["bvd"][:])
    for sc in range(1, 4):
        nc.sync.dma_start(out=src_sb[:, :, 1024 * sc:1024 * (sc + 1)],
                          in_=srcr[:, :, 1024 * sc:1024 * (sc + 1)])
    nc.sync.dma_start(out=wm_sb[:], in_=io["wmT"].rearrange("(c p) o -> p c o", p=128))
    nc.sync.dma_start(out=bm_sb[:], in_=io["bm"][:])
    nc.sync.dma_start(out=w1x_sb[:], in_=io["w1xT"].rearrange("(c p) o -> p c o", p=128))
    nc.sync.dma_start(out=w1m_sb[:], in_=io["w1mT"].rearrange("(c p) o -> p c o", p=128))
    nc.sync.dma_start(out=b1_sb[:], in_=io["b1"][:])
    nc.sync.dma_start(out=w2_sb[:], in_=io["w2T"].rearrange("(c p) o -> p c o", p=128))
    nc.sync.dma_start(out=b2_sb[:], in_=io["b2"][:])

    nc.vector.memset(vaT_sb[:, :, :, :, DH:DH + 1], 1.0)
    nc.gpsimd.partition_broadcast(bvb_sb[:], bv_sb[:])

    # ---------- filler quanta (PE work interleaved into the phase loop) ----
    def f_q(oc, nsb):
        def fn(mm):
            ps = mm.tile([128, 512], FP, tag="mm")
            for ic in range(2):
                nc.tensor.matmul(ps[:], wq_sb[:, ic, oc * 128:(oc + 1) * 128],
                                 xs_sb[:, ic, nsb * 512:(nsb + 1) * 512],
                                 start=(ic == 0), stop=(ic == 1))
            nc.vector.tensor_scalar_add(
                q_sb[:, oc, nsb * 512:(nsb + 1) * 512], ps[:], bq_sb[:, oc:oc + 1])
        return fn

    def f_k(oc, half):
        def fn(mm):
            ps = mm.tile([128, 512], FP, tag="mm")
            for ic in range(2):
                nc.tensor.matmul(ps[:], wk_sb[:, ic, oc * 128:(oc + 1) * 128],
                                 src_sb[:, ic, half * 512:(half + 1) * 512],
                                 start=(ic == 0), stop=(ic == 1))
            nc.vector.tensor_scalar_add(
                k_sb[:, oc, half * 512:(half + 1) * 512], ps[:], bk_sb[:, oc:oc + 1])
        return fn

    def f_v(pairs):
        def fn(mm):
            # v^T for m columns of the given pair-chunks; bias added via the
            # partition-broadcast bvb during the PSUM eviction.
            for pair in pairs:
                ps = mm.tile([128, 2, H, DH], FP, tag="mm", name=f"vps{pair}")
                for par in range(2):
                    mc = 2 * pair + par
                    for ic in range(2):
                        nc.tensor.matmul(ps[:, par],
                                         src_sb[:, ic, mc * 128:(mc + 1) * 128],
                                         wv_sb[:, ic, :],
                                         start=(ic == 0), stop=(ic == 1))
                nc.vector.tensor_tensor(out=vaT_sb[:, pair, :, :, 0:DH],
                                        in0=ps[:], in1=bvb_sb[:], op=OP.add)
        return fn

    def f_msg(b, oc):
        def fn(mm):
            n0 = b * W0
            ps = mm.tile([128, 512], FP, tag="mm")
            for ic in range(2):
                nc.tensor.matmul(ps[:], wm_sb[:, ic, oc * 128:(oc + 1) * 128],
                                 attn_sb[:, ic, n0:n0 + W0],
                                 start=(ic == 0), stop=(ic == 1))
            nc.vector.tensor_scalar_add(
                msg_sb[:, oc, n0:n0 + W0], ps[:], bm_sb[:, oc:oc + 1])
        return fn

    def f_h1(b, t):
        def fn(mm):
            n0 = b * W0
            ps = mm.tile([128, 512], FP, tag="mm")
            for ic in range(2):
                nc.tensor.matmul(ps[:], w1x_sb[:, ic, t * 128:(t + 1) * 128],
                                 xs_sb[:, ic, n0:n0 + W0], start=(ic == 0), stop=False)
            for ic in range(2):
                nc.tensor.matmul(ps[:], w1m_sb[:, ic, t * 128:(t + 1) * 128],
                                 msg_sb[:, ic, n0:n0 + W0], start=False, stop=(ic == 1))
            nc.vector.tensor_scalar_add(h1_sb[:, t, n0:n0 + W0], ps[:],
                                        b1_sb[:, t:t + 1])
            nc.vector.bn_stats(out=bst_sb[:, t, b, :], in_=h1_sb[:, t, n0:n0 + W0])
        return fn

    # ---------- scores / exp / attn emitters ----------
    def emit_group(b, hp, parity, gi, scA, scB):
        j0, glen = JGROUPS[gi]
        n0 = b * W0
        pool, tag = (scA, "scA") if glen == 2 else (scB, "scB")
        sc_ps = pool.tile([128, 2, glen, W0], FP, tag=tag)
        for j4 in range(glen):
            jj = j0 + j4
            for hh in range(2):
                bp = 64 * hh
                nc.tensor.matmul(
                    sc_ps[:, hh, j4, :],
                    k_sb[bp:bp + DH, hp, jj * 128:(jj + 1) * 128],
                    q_sb[bp:bp + DH, hp, n0:n0 + W0],
                    start=True, stop=True)
        nc.scalar.activation(
            out=exp_sb[:, parity, :, j0:j0 + glen, :], in_=sc_ps[:],
            func=AF.Exp, scale=0.125)

    def attn_bundles(b, hp, parity):
        """List of emit-closures: 4 bundles of 4 DR passes + a norm, per hh."""
        n0 = b * W0
        state = {}

        def passes(hh, p0):
            def fn(atp):
                h = 2 * hp + hh
                if p0 == 0:
                    state[hh] = atp.tile([128, W0], FP, tag=f"at{hh}",
                                         name=f"at{hh}_{b}_{hp}")
                at_ps = state[hh]
                for p in range(p0, p0 + 4):
                    nc.tensor.matmul(
                        at_ps[:DH + 1, :],
                        vaT_sb[:, p, :, h, 0:DH + 1],
                        exp_sb[:, parity, hh, 2 * p:2 * p + 2, :],
                        start=(p == 0), stop=(p == 15),
                        perf_mode=DR)
            return fn

        def norm(hh):
            def fn(atp):
                bp = 64 * hh
                at_ps = state[hh]
                rz = npool.tile([1, W0], FP, tag="rz")
                nc.vector.reciprocal(rz[:], at_ps[DH:DH + 1, :])
                rzb = npool.tile([DH, W0], FP, tag="rzb")
                nc.gpsimd.partition_broadcast(rzb[:], rz[:])
                nc.vector.tensor_mul(
                    attn_sb[bp:bp + DH, hp, n0:n0 + W0], at_ps[0:DH, :], rzb[:])
            return fn

        out = []
        for hh in range(2):
            out += [passes(hh, p0) for p0 in (0, 4, 8, 12)]
            out.append(norm(hh))
        return out

    # ---------- pre-phase: q/k needed by the first score groups ----------
    with tc.tile_pool(name="pre", bufs=2, space="PSUM") as pre:
        f_q(0, 0)(pre)
        f_k(0, 0)(pre)

    if _STAGE == "proj":
        with tc.tile_pool(name="pp", bufs=2, space="PSUM") as pp:
            f_q(0, 1)(pp)
            f_q(1, 0)(pp)
            f_q(1, 1)(pp)
            for hf in range(1, 8):
                f_k(0, hf)(pp)
            for hf in range(8):
                f_k(1, hf)(pp)
            for pair in range(16):
                f_v([pair])(pp)
        nc.vector.tensor_copy(out_sb[:], q_sb[:])
        nc.sync.dma_start(out=io["out"].rearrange("(c p) n -> p c n", p=128),
                          in_=out_sb[:])
        return

    # ---------- per-phase filler schedules (consumed one per mm slot; the
    # first is emitted before group g2 so scores never chase their own deps).
    # k half h feeds the score group covering m-chunk 4h (need-by slots:
    # h1->g2, h2->g5, h3->g8, h4->g10, h5->g13, h6->g16, h7->g18).
    phase_fill = {
        0: [f_k(0, 1), f_k(0, 2), f_q(1, 0), f_q(1, 1), f_k(1, 0), f_k(0, 3),
            f_k(1, 1), f_k(0, 4), f_q(0, 1), f_k(0, 5), f_v([0, 1]), f_k(0, 6),
            f_v([2, 3]), f_k(0, 7), f_v([4, 5])],
        1: [f_k(1, 2), f_k(1, 3), f_k(1, 4), f_k(1, 5), f_k(1, 6), f_k(1, 7),
            f_v([6, 7]), f_v([8, 9]), f_v([10, 11]), f_v([12, 13]),
            f_v([14, 15])],
        2: [],
        3: [f_msg(0, 0), f_msg(0, 1)] + [f_h1(0, t) for t in range(4)],
    }
    MMEND = {0: 17, 1: 17, 2: 17, 3: 13}

    phases = [(b, hp) for b in range(NBLK) for hp in range(2)]
    with tc.tile_pool(name="scA", bufs=1, space="PSUM") as scA, \
         tc.tile_pool(name="scB", bufs=1, space="PSUM") as scB:
        for pi, (b, hp) in enumerate(phases):
            parity = pi % 2
            emit_group(b, hp, parity, 0, scA, scB)
            emit_group(b, hp, parity, 1, scA, scB)
            work = list(phase_fill[pi])
            with tc.tile_pool(name="mm", bufs=2, space="PSUM") as mm:
                for gi in range(2, MMEND[pi]):
                    if work:
                        work.pop(0)(mm)
                    emit_group(b, hp, parity, gi, scA, scB)
                while work:
                    work.pop(0)(mm)
            with tc.tile_pool(name="at", bufs=1, space="PSUM") as atp:
                bundles = []
                if pi > 0:
                    pb, php = phases[pi - 1]
                    bundles = attn_bundles(pb, php, (pi - 1) % 2)
                ngroups = NG - MMEND[pi]
                per = (len(bundles) + ngroups - 1) // ngroups if bundles else 0
                for gi in range(MMEND[pi], NG):
                    emit_group(b, hp, parity, gi, scA, scB)
                    for _ in range(per):
                        if bundles:
                            bundles.pop(0)(atp)
                while bundles:
                    bundles.pop(0)(atp)
                if pi == len(phases) - 1:
                    # own attn interleaved right behind the exp stream
                    for fn in attn_bundles(b, hp, parity):
                        fn(atp)

    if _STAGE == "attn":
        nc.vector.tensor_copy(out_sb[:], attn_sb[:])
        nc.sync.dma_start(out=io["out"].rearrange("(c p) n -> p c n", p=128),
                          in_=out_sb[:])
        return

    # ---------- tail: mlp of the last block, stats, collective, output ----
    with tc.tile_pool(name="mm", bufs=2, space="PSUM") as mm:
        for fn in [f_msg(1, 0), f_msg(1, 1)] + [f_h1(1, t) for t in range(4)]:
            fn(mm)

        if _STAGE == "h1":
            nc.vector.tensor_copy(out_sb[:, 0, :], h1_sb[:, 0, :])
            nc.vector.tensor_copy(out_sb[:, 1, :], h1_sb[:, 1, :])
            nc.sync.dma_start(out=io["out"].rearrange("(c p) n -> p c n", p=128),
                              in_=out_sb[:])
            return

        with tc.tile_pool(name="dram", bufs=1, space="DRAM") as dram:
            for t in range(4):
                mv = npool.tile([128, 2], FP, tag="mv")
                nc.vector.bn_aggr(out=mv[:], in_=bst_sb[:, t, :, :])
                nc.vector.tensor_scalar_mul(stats_sb[:, t:t + 1], mv[:, 0:1], float(NS))
                msq = npool.tile([128, 1], FP, tag="msq")
                nc.vector.tensor_mul(msq[:], mv[:, 0:1], mv[:, 0:1])
                msq2 = npool.tile([128, 1], FP, tag="msq2")
                nc.vector.tensor_add(msq2[:], mv[:, 1:2], msq[:])
                nc.vector.tensor_scalar_mul(stats_sb[:, 4 + t:5 + t], msq2[:], float(NS))

            sred = npool.tile([128, 4, 8], FP, tag="sred")
            if _STAGE == "nocc":
                for c in range(4):
                    nc.vector.tensor_copy(sred[:, c, :], stats_sb[:])
            else:
                cc_in = dram.tile([128, 8], FP)
                cc_out = dram.tile([4, 128, 8], FP)
                nc.sync.dma_start(out=cc_in[:], in_=stats_sb[:])
                nc.gpsimd.collective_compute(
                    "AllGather", OP.bypass,
                    replica_groups=[[0, 1, 2, 3], [4, 5, 6, 7]],
                    ins=[cc_in[:].opt()], outs=[cc_out[:].opt()],
                )
                nc.sync.dma_start(out=sred[:], in_=cc_out[:].rearrange("c p s -> p c s"))

            s01 = npool.tile([128, 8], FP, tag="s01")
            nc.vector.tensor_add(s01[:], sred[:, 0, :], sred[:, 1, :])
            stot = npool.tile([128, 8], FP, tag="stot")
            nc.vector.tensor_add(stot[:], sred[:, 2, :], sred[:, 3, :])
            nc.vector.tensor_tensor(out=stot[:], in0=s01[:], in1=stot[:], op=OP.add)

            scl = npool.tile([128, 8], FP, tag="scl")
            nc.vector.tensor_scalar_mul(scl[:], stot[:], 1.0 / N)  # [mu | E x^2]
            var4 = npool.tile([128, 4], FP, tag="var4")
            nc.vector.tensor_mul(var4[:], scl[:, 0:4], scl[:, 0:4])
            nc.vector.tensor_tensor(out=var4[:], in0=scl[:, 4:8], in1=var4[:],
                                    op=OP.subtract)
            eps1 = npool.tile([128, 1], FP, tag="eps1")
            nc.vector.memset(eps1[:], EPS)
            std4 = npool.tile([128, 4], FP, tag="std4")
            nc.scalar.activation(out=std4[:], in_=var4[:], func=AF.Sqrt, bias=eps1[:])
            rstd4 = npool.tile([128, 4], FP, tag="rstd4")
            nc.vector.reciprocal(rstd4[:], std4[:])
            nb4 = npool.tile([128, 4], FP, tag="nb4")
            nc.vector.scalar_tensor_tensor(
                out=nb4[:], in0=scl[:, 0:4], scalar=-1.0, in1=rstd4[:],
                op0=OP.mult, op1=OP.mult)

        # ---------- relu + W2 interleaved per contraction chunk ----------
        outr = io["out"].rearrange("(c p) n -> p c n", p=128)
        for nsb in range(2):
            o_ps = [mm.tile([128, 512], FP, tag="mm", name=f"ops{nsb}_{oc}")
                    for oc in range(2)]
            for t in range(4):
                nc.scalar.activation(
                    out=h1n_sb[:, t, nsb * 512:(nsb + 1) * 512],
                    in_=h1_sb[:, t, nsb * 512:(nsb + 1) * 512], func=AF.Relu,
                    bias=nb4[:, t:t + 1], scale=rstd4[:, t:t + 1])
                for oc in range(2):
                    nc.tensor.matmul(
                        o_ps[oc][:],
                        w2_sb[:, t, oc * 128:(oc + 1) * 128],
                        h1n_sb[:, t, nsb * 512:(nsb + 1) * 512],
                        start=(t == 0), stop=(t == 3))
            for oc in range(2):
                nc.vector.tensor_scalar_add(
                    out_sb[:, oc, nsb * 512:(nsb + 1) * 512], o_ps[oc][:],
                    b2_sb[:, oc:oc + 1])
            nc.sync.dma_start(out=outr[:, :, nsb * 512:(nsb + 1) * 512],
                              in_=out_sb[:, :, nsb * 512:(nsb + 1) * 512])


_BUILT = {}


def _build():
    if "nc" in _BUILT:
        return _BUILT["nc"]
    nc = bacc.Bacc("TRN2", target_bir_lowering=False, debug=False,
                   enable_asserts=True, num_devices=NCORES)
    io = {}
    io["xs"] = nc.dram_tensor("xs", [D, NS], BF, kind="ExternalInput").ap()
    io["src"] = nc.dram_tensor("src", [D, N], BF, kind="ExternalInput").ap()
    io["wqT"] = nc.dram_tensor("wqT", [D, D], BF, kind="ExternalInput").ap()
    io["wkT"] = nc.dram_tensor("wkT", [D, D], BF, kind="ExternalInput").ap()
    io["wvT"] = nc.dram_tensor("wvT", [D, D], BF, kind="ExternalInput").ap()
    io["wmT"] = nc.dram_tensor("wmT", [D, D], BF, kind="ExternalInput").ap()
    io["w1xT"] = nc.dram_tensor("w1xT", [D, 2 * D], BF, kind="ExternalInput").ap()
    io["w1mT"] = nc.dram_tensor("w1mT", [D, 2 * D], BF, kind="ExternalInput").ap()
    io["w2T"] = nc.dram_tensor("w2T", [2 * D, D], BF, kind="ExternalInput").ap()
    io["bq"] = nc.dram_tensor("bq", [128, 2], FP, kind="ExternalInput").ap()
    io["bk"] = nc.dram_tensor("bk", [128, 2], FP, kind="ExternalInput").ap()
    io["bvd"] = nc.dram_tensor("bvd", [1, 2 * D], FP, kind="ExternalInput").ap()
    io["bm"] = nc.dram_tensor("bm", [128, 2], FP, kind="ExternalInput").ap()
    io["b1"] = nc.dram_tensor("b1", [128, 4], FP, kind="ExternalInput").ap()
    io["b2"] = nc.dram_tensor("b2", [128, 2], FP, kind="ExternalInput").ap()
    io["out"] = nc.dram_tensor("out", [D, NS], FP, kind="ExternalOutput").ap()

    import contextlib
    with tile.TileContext(nc) as tc:
        with contextlib.ExitStack() as es:
            _emit(nc, tc, io, es)
    nc.compile()
    _BUILT["nc"] = nc
    return nc


def _prep_inputs(x, source, Wq, bq, Wk, bk, Wv, bv, Wm, bm, W1, b1, W2, b2):
    perm = np.array([4 * d + h for h in range(H) for d in range(DH)])
    f32 = lambda a: np.ascontiguousarray(a, dtype=np.float32)
    bf = lambda a: np.ascontiguousarray(np.asarray(a, dtype=np.float32),
                                        ).astype(ml_dtypes.bfloat16)

    shared = {
        "wqT": bf(Wq[perm, :].T),
        "wkT": bf(Wk[perm, :].T),
        "wvT": bf(Wv[perm, :].T),
        "wmT": bf(Wm[:, perm].T),
        "w1xT": bf(W1.T[0:D, :]),
        "w1mT": bf(W1.T[D:2 * D, :]),
        "w2T": bf(W2.T),
        "bq": f32(bq[perm].reshape(2, 128).T),
        "bk": f32(bk[perm].reshape(2, 128).T),
        "bvd": f32(np.concatenate([bv[perm], bv[perm]]).reshape(1, 2 * D)),
        "bm": f32(bm.reshape(2, 128).T),
        "b1": f32(b1.reshape(4, 128).T),
        "b2": f32(b2.reshape(2, 128).T),
    }
    in_maps = []
    for core in range(NCORES):
        b, s = core // 4, core % 4
        m = dict(shared)
        m["xs"] = bf(x[b][:, s * NS:(s + 1) * NS])
        m["src"] = bf(source[b])
        in_maps.append(m)
    return in_maps


def run(inputs, **spmd_kwargs):
    """Build (cached), run on cores 0-7, return (full_output, BassKernelResults)."""
    nc = _build()
    in_maps = _prep_inputs(**inputs)
    res = bass_utils.run_bass_kernel_spmd(
        nc, in_maps, core_ids=list(range(NCORES)), **spmd_kwargs)
    full = np.empty((B, D, N), dtype=np.float32)
    for core in range(NCORES):
        b, s = core // 4, core % 4
        full[b][:, s * NS:(s + 1) * NS] = res.results[core]["out"]
    return full, res


def kernel(**inputs):
    full, _ = run(inputs)
    return full
